# revision 49
# baseline (speedup 1.0000x reference)
"""AttnBlock (GroupNorm -> QKV 1x1 conv -> spatial attention with softmax over
query-H axis -> output projection + residual) for B=8, C=128, H=W=48 on 8
Trainium2 NeuronCores, data-parallel over batch (1 batch per core).

Math per batch (N = H*W = 2304 spatial positions, C = 128 channels):
  xn = GroupNorm(x; 32 groups of 4 channels)
  q/k/v = W @ xn + b              (per-position 1x1 conv = C x C matmul)
  S[q', kp] = q[:,q'] . k[:,kp] / sqrt(C)
  attn = softmax over the query-H axis: for fixed (w, kp), normalize over h
  ctx[c, (h,w)] = sum_kp attn[(h,w), kp] * v[c, kp]
  out = x + Wo @ ctx + bo

Device mapping (natural layout + paired all-VectorE fold-chain softmax):
  - Channels on the 128 SBUF partitions; spatial positions on the free axis
    in NATURAL (h, w) order (q' = h*48 + w).  Softmax groups (fixed w,
    varying h) are stride-48; all softmax arithmetic keeps a contiguous
    step-1 inner axis (w), which the DVE needs for its 2x bf16 mode
    (HW-verified: outer-stride jumps are fine, inner broadcasts/strides
    fall to 1x or worse).
  - S computed transposed (S^T [kp, q']) per 128-key chunk; exp on ScalarE
    at 768 granularity (PSUM bank limit); E chunks live in SBUF (bf16), two
    chunks per tile so softmax post-processing runs once per PAIR of chunks
    (halves per-op DVE/semaphore overhead, ~0.3us per op).
  - Softmax denominators WITHOUT tensor_reduce (capped at 1x, 2.4us/chunk):
    a fold chain over h (48 = 3x16, then halves to 1) of in-place bf16
    2x-mode tensor_tensor adds, ~1.4us/chunk, entirely on VectorE.
    GpSimd is NOT used at all: its concurrent SBUF traffic was measured to
    stall DVE ops ~3x, a worse trade than doing everything on VectorE.
  - Normalize-mul for both chunks of a pair in one 2x DVE op ([p,c,h,w] *=
    rden[p,c,w] broadcast over the OUTER h axis); reciprocal->bf16 cast on
    ScalarE (spare capacity there).
  - ctx accumulates in 4 PSUM banks for columns 0:2048 (lagged four chunks
    behind the softmax chain); the 256-column tail gets a short dense pass
    at the end, between output-projection groups in the TensorE queue.
  - GroupNorm statistics via bn_stats segments pipelined with the x DMA
    (mean/var -> group-combine with tiny matmuls); affine folded into the
    projection weights.  All [C,*] constants arrive in ONE packed DMA blob
    (DMA issue on the sync queue costs ~0.6us per descriptor).
  - Ln activation table warmed at kernel start so its load overlaps DMA;
    dummy matmuls (a burst + ticks gated on prologue intermediates) keep
    the PE HAM clock-gate warm through the prologue.
  - Output projection evacuation fused with bias + residual add in a single
    VectorE scalar_tensor_tensor per 768 columns; output DMA split per
    768-column group.
"""

import sys

sys.path.insert(0, "/opt/trn_rl_repo")

import numpy as np

import concourse.bass as bass
import concourse.mybir as mybir
import concourse.tile as tile
from concourse import bacc, bass_utils

B, C, H, W = 8, 128, 48, 48
N = H * W  # 2304
GROUPS = 32
GSIZE = C // GROUPS
EPS = 1e-5
NCORES = 8

F32 = mybir.dt.float32
F32R = mybir.dt.float32r
BF16 = mybir.dt.bfloat16
AF = mybir.ActivationFunctionType
OP = mybir.AluOpType

NCHUNK = N // 128  # 18 key chunks
NPAIR = NCHUNK // 2  # 9 softmax pairs
QG = 768  # S^T staging / exp granularity
NQG = N // QG  # 3
CTX_LIVE = [0, 512, 1024, 1536]  # 4 psum-resident ctx banks (512 wide each)
TAIL_OFF, TAIL_SZ = 2048, 256  # final ctx region, computed in a tail pass


def _build_program():
    # Force Ln AND Exp to resolve to the combined natural_log_exp_and_others
    # table set (one ACT_TABLE_LOAD instead of two, and no mid-prologue
    # reload).  Blanking the single-function sets keeps every set at its
    # original act_info.json index, so emitted act_func_set_ids stay valid.
    _orig_tables = bacc.get_activation_tables

    def _tables_combined(arch):
        t = dict(_orig_tables(arch))
        for name in ("exp_and_others", "natural_log", "exp_and_friends"):
            if name in t:
                t[name] = set()
        return t

    bacc.get_activation_tables = _tables_combined
    try:
        return _build_program_inner()
    finally:
        bacc.get_activation_tables = _orig_tables


def _build_program_inner():
    nc = bacc.Bacc("TRN2", target_bir_lowering=False, debug=False)

    def din(name, shape, dt=F32):
        return nc.dram_tensor(name, shape, dt, kind="ExternalInput")

    x_d = din("x", [C, N], F32R)
    # all [C, *] constants packed into one blob: 4 weights (f32r), gmat,
    # ident (bf16 as 64 f32 cols), gn_w/gn_b/bq/bk/bv/bo
    CB = 4 * C + GROUPS + C // 2 + 6
    cblob_d = din("cblob", [C, CB], F32R)
    gexp_d = din("gexp", [GROUPS, C], F32R)
    woTb_d = din("woTb", [C, C], BF16)
    out_d = nc.dram_tensor("out", [C, N], F32, kind="ExternalOutput")

    NSEG = 6  # x DMA / bn_stats segments
    SEG = N // NSEG  # 384

    with tile.TileContext(nc) as tc:
        with (
            tc.tile_pool(name="const", bufs=1) as const,
            tc.tile_pool(name="data", bufs=1) as data,
            tc.tile_pool(name="small", bufs=1) as small,
            tc.tile_pool(name="soft", bufs=4) as soft,
            tc.tile_pool(name="epool", bufs=NPAIR) as epool,
        ):
            # ---- warm the Ln activation table while DMAs run (Exp lives in a
            # ---- different set; its load lands after the GroupNorm Ln) ----
            warm = small.tile([C, 2], F32)
            nc.vector.memset(warm[:], 1.0)
            nc.scalar.activation(warm[:, 1:2], warm[:, 1:2], AF.Ln)

            # ---- warm the PE-array HAM clock gate: a short burst flips the
            # ---- clock 1.2->2.4 GHz (~3.4us of sustained activity); later,
            # ---- tiny dummy matmuls gated on prologue intermediates keep
            # ---- every ~3.4us HAM window non-idle until real matmuls flow
            # ---- (run-to-run timing otherwise varies ~20% on HAM phase) ----
            warmps_ctx = tc.tile_pool(name="warmps", bufs=1, space="PSUM")
            warmps = warmps_ctx.__enter__()
            wmt = small.tile([C, 512], BF16)
            nc.gpsimd.memset(wmt[:], 0.0)
            wps = warmps.tile([C, 512], F32)
            for _ in range(10):
                nc.tensor.matmul(
                    wps[:], wmt[:, 0:128], wmt[:], start=True, stop=True
                )

            def ham_tick(dep_ap):
                """Tiny dummy matmul whose rhs aliases `dep_ap` (bitcast to
                bf16) so it runs right after that producer — cheap PE-queue
                activity spaced through the serial prologue chain."""
                rhs = dep_ap.bitcast(BF16)
                cols = min(rhs.shape[-1], 64)
                nc.tensor.matmul(
                    wps[:, 0:cols], wmt[:, 0:128], rhs[:, 0:cols],
                    start=True, stop=True,
                )

            # ---- input loads (x first, segmented: GroupNorm stats overlap) ----
            tx = data.tile([C, N], F32R)
            for i in range(NSEG):
                nc.sync.dma_start(
                    tx[:, SEG * i : SEG * (i + 1)], x_d[:, SEG * i : SEG * (i + 1)]
                )
                ham_tick(tx[:, SEG * i : SEG * i + 64])
            txf = tx[:].bitcast(F32)
            # bf16 copy of x for the (cheaper) bf16 projection matmuls
            tx_bf = data.tile([C, N], BF16)
            for i in range(3):
                nc.vector.tensor_copy(
                    tx_bf[:, 768 * i : 768 * (i + 1)],
                    txf[:, 768 * i : 768 * (i + 1)],
                )

            cblob = const.tile([C, CB], F32R)
            gexp = const.tile([GROUPS, C], F32R)
            woTb = const.tile([C, C], BF16)
            nc.sync.dma_start(cblob[:], cblob_d[:])
            nc.sync.dma_start(gexp[:], gexp_d[:])
            nc.sync.dma_start(woTb[:], woTb_d[:])
            wqT = cblob[:, 0:C]
            wkT = cblob[:, C : 2 * C]
            wvT = cblob[:, 2 * C : 3 * C]
            woT = cblob[:, 3 * C : 4 * C]
            gmat = cblob[:, 4 * C : 4 * C + GROUPS]
            ident = cblob[:, 4 * C + GROUPS : 4 * C + GROUPS + C // 2].bitcast(BF16)
            bof = 4 * C + GROUPS + C // 2
            cbf = cblob[:].bitcast(F32)
            gnw = cbf[:, bof : bof + 1]
            gnb = cbf[:, bof + 1 : bof + 2]
            bq = cbf[:, bof + 2 : bof + 3]
            bk = cbf[:, bof + 3 : bof + 4]
            bv = cbf[:, bof + 4 : bof + 5]
            bo = cbf[:, bof + 5 : bof + 6]

            # ---- GroupNorm statistics: bn_stats per DMA segment ----
            bstats = small.tile([C, NSEG, 6], F32)
            for i in range(NSEG):
                nc.vector.bn_stats(
                    bstats[:, i, :], txf[:, SEG * i : SEG * (i + 1)]
                )
                ham_tick(bstats[:, i, :])
            mv = small.tile([C, 2], F32)  # per-channel (mean, var)
            nc.vector.bn_aggr(mv[:], bstats[:].rearrange("p g f -> p (g f)"))

            # per-channel (mean, E[x^2]); group-combine via gmat matmul
            mex = small.tile([C, 2], F32)
            nc.vector.tensor_mul(mex[:, 1:2], mv[:, 0:1], mv[:, 0:1])
            nc.vector.tensor_add(mex[:, 1:2], mex[:, 1:2], mv[:, 1:2])
            nc.vector.tensor_copy(mex[:, 0:1], mv[:, 0:1])
            mex_r = small.tile([C, 2], F32R)
            nc.vector.tensor_copy(mex_r[:], mex[:])

            with tc.tile_pool(name="gnps", bufs=1, space="PSUM") as gnps:
                psg = gnps.tile([GROUPS, 2], F32)
                nc.tensor.matmul(psg[:], gmat, mex_r[:], start=True, stop=True)

                inv_g = 1.0 / GSIZE
                t32 = small.tile([GROUPS, 4], F32)
                nc.vector.tensor_scalar_mul(t32[:, 0:1], psg[:, 0:1], inv_g)
                nc.vector.tensor_scalar_mul(t32[:, 1:2], psg[:, 1:2], inv_g)
                nc.vector.tensor_mul(t32[:, 2:3], t32[:, 0:1], t32[:, 0:1])
                nc.vector.tensor_sub(t32[:, 3:4], t32[:, 1:2], t32[:, 2:3])
                eps_t = small.tile([GROUPS, 1], F32)
                nc.vector.memset(eps_t[:], EPS)
                nc.scalar.activation(t32[:, 2:3], t32[:, 3:4], AF.Ln, bias=eps_t[:])
                rstd_f = small.tile([GROUPS, 1], F32)
                nc.scalar.activation(rstd_f[:], t32[:, 2:3], AF.Exp, scale=-0.5)
                mstat = small.tile([GROUPS, 2], F32R)
                nc.vector.tensor_copy(mstat[:, 0:1], t32[:, 0:1])
                nc.vector.tensor_copy(mstat[:, 1:2], rstd_f[:])

                pse = gnps.tile([C, 2], F32)
                nc.tensor.matmul(pse[:], gexp[:], mstat[:], start=True, stop=True)

                A_sb = small.tile([C, 1], F32)
                B_sb = small.tile([C, 1], F32)
                nc.vector.tensor_mul(A_sb[:], pse[:, 1:2], gnw)
                nc.vector.tensor_mul(B_sb[:], pse[:, 0:1], A_sb[:])
                nc.vector.tensor_sub(B_sb[:], gnb, B_sb[:])
                ham_tick(B_sb[:])

            # ---- fold the GroupNorm affine into the projection weights:
            # ---- q = Wq(A*x + B) + bq = (Wq diag(A)) x + (Wq B + bq)
            wq2 = small.tile([C, C], BF16)
            wk2 = small.tile([C, C], BF16)
            wv2 = small.tile([C, C], BF16)
            bq2 = small.tile([C, 1], F32)
            bk2 = small.tile([C, 1], F32)
            bv2 = small.tile([C, 1], F32)
            with tc.tile_pool(name="foldps", bufs=1, space="PSUM") as foldps:
                psb = foldps.tile([C, 4], F32)
                for i, (wT, w2, bias, b2) in enumerate((
                    (wqT, wq2, bq, bq2),
                    (wkT, wk2, bk, bk2),
                    (wvT, wv2, bv, bv2),
                )):
                    nc.vector.tensor_scalar_mul(w2[:], wT.bitcast(F32), A_sb[:])
                    nc.tensor.matmul(
                        psb[:, i : i + 1], wT.bitcast(F32), B_sb[:],
                        start=True, stop=True,
                    )
                    nc.vector.tensor_add(b2[:], psb[:, i : i + 1], bias)

            # ---- Q/K/V projections + attention with STAGED PSUM pools:
            # ---- phase A (projps+sps): q/k/v projections, S^T chunks 0-1;
            # ---- phase B (pvtps+sps): v transposes, S^T chunks 2-3;
            # ---- phase C (ctxps+sps): the AV-accumulating main loop.
            # ---- This puts the first S^T right behind the q/k matmuls in
            # ---- the TensorE queue instead of behind all 36 proj+transpose
            # ---- matmuls, and hides the transposes under early softmax.
            q = data.tile([C, N], BF16)
            k = data.tile([C, N], BF16)
            v = data.tile([C, N], BF16)
            vT = data.tile([C, NCHUNK * C], BF16)
            pair_tiles = [None] * NPAIR
            e_tiles = [None] * NCHUNK
            ctx_all = data.tile([C, N], BF16)
            ctx_ps = []

            def emit_av(ch, part):
                ec = e_tiles[ch]
                for i in ([0, 1], [2], [3])[part]:
                    o = CTX_LIVE[i]
                    nc.tensor.matmul(
                        ctx_ps[i][:, :],
                        vT[:, 128 * ch : 128 * (ch + 1)],
                        ec[:, o : o + 512],
                        start=(ch == 0),
                        stop=(ch == NCHUNK - 1),
                    )

            warmps_ctx.__exit__(None, None, None)

            with tc.tile_pool(name="sps", bufs=2, space="PSUM") as sps:

                def proj_group(pool, wT, bias, dst, eng, g):
                    pp = pool.tile([C, QG], F32, tag="pp")
                    o = g * QG
                    nc.tensor.matmul(
                        pp[:, 0:512], wT[:], tx_bf[:, o : o + 512],
                        start=True, stop=True,
                    )
                    nc.tensor.matmul(
                        pp[:, 512:QG], wT[:], tx_bf[:, o + 512 : o + QG],
                        start=True, stop=True,
                    )
                    outv = dst[:, o : o + QG]
                    if eng == "scalar":
                        nc.scalar.activation(outv, pp[:], AF.Identity, bias=bias[:])
                    else:
                        nc.vector.tensor_scalar(outv, pp[:], bias[:], None, op0=OP.add)

                def attn_iter(it):
                    ch = it if it < NCHUNK else None
                    av = it - 4
                    if ch is not None:
                        pj = ch // 2
                        if ch % 2 == 0:
                            pair_tiles[pj] = epool.tile(
                                [C, 2, N], BF16, tag="E", name=f"E_{pj}"
                            )
                        ec = pair_tiles[pj][:, ch % 2, :]
                        e_tiles[ch] = ec
                        klhs = k[:, 128 * ch : 128 * (ch + 1)]
                        for g in range(NQG):
                            ps = sps.tile([C, QG], F32, tag="spsum")
                            o = g * QG
                            # dummy matmul into the about-to-be-overwritten
                            # sps region: raises PE duty past the HAM
                            # clock-gate threshold so the loop stays at 2.4GHz
                            nc.tensor.matmul(
                                ps[:, 0:512], wmt[:, 0:128], wmt[:],
                                start=True, stop=True,
                            )
                            nc.tensor.matmul(
                                ps[:, 0:512], klhs, q[:, o : o + 512],
                                start=True, stop=True,
                            )
                            nc.tensor.matmul(
                                ps[:, 512:QG], klhs, q[:, o + 512 : o + QG],
                                start=True, stop=True,
                            )
                            nc.scalar.activation(ec[:, o : o + QG], ps[:, :], AF.Exp)
                            if av >= 0:
                                emit_av(av, g)
                    else:
                        # duty dummy per drain iteration: keeps the HAM
                        # clock-gate warm through the AV drain + epilogue
                        ps_d = sps.tile([C, QG], F32, tag="spsum")
                        nc.tensor.matmul(
                            ps_d[:, 0:512], wmt[:, 0:128], wmt[:],
                            start=True, stop=True,
                        )
                        for g in range(NQG):
                            emit_av(av, g)

                    if ch is None or ch % 2 == 0:
                        return
                    # ---- softmax denominators + normalize, once per pair ----
                    # fold chain over h (48 = 3x16, then halves): every stage
                    # is a contiguous bf16 2x-mode add (inner axis w step-1);
                    # no tensor_reduce (1x-capped), no GpSimd (its SBUF
                    # traffic stalls concurrent DVE ops ~3x), no cross-engine
                    # hops inside the chain.
                    pj = ch // 2
                    ep4 = pair_tiles[pj][:].rearrange(
                        "p c (h w) -> p c h w", h=H
                    )
                    fold = soft.tile([C, 2, 16, W], BF16, tag="F")
                    nc.vector.tensor_tensor(
                        out=fold[:], in0=ep4[:, :, 0:16, :],
                        in1=ep4[:, :, 16:32, :], op=OP.add,
                    )
                    nc.vector.tensor_tensor(
                        out=fold[:], in0=fold[:],
                        in1=ep4[:, :, 32:48, :], op=OP.add,
                    )
                    for hh in (8, 4, 2):
                        nc.vector.tensor_tensor(
                            out=fold[:, :, 0:hh, :], in0=fold[:, :, 0:hh, :],
                            in1=fold[:, :, hh : 2 * hh, :], op=OP.add,
                        )
                    dsum = soft.tile([C, 2, W], F32, tag="D")
                    nc.vector.tensor_tensor(
                        out=dsum[:], in0=fold[:, :, 0, :],
                        in1=fold[:, :, 1, :], op=OP.add,
                    )
                    rden = soft.tile([C, 2 * W], F32, tag="R")
                    nc.vector.reciprocal_approx_fast(
                        rden[:], dsum[:].rearrange("p c w -> p (c w)")
                    )
                    rden_b = soft.tile([C, 2, W], BF16, tag="Rb")
                    nc.scalar.copy(
                        rden_b[:].rearrange("p c w -> p (c w)"), rden[:]
                    )
                    # normalize both chunks in one 2x DVE op:
                    # [p, c, h, w] *= rden_b[p, c, w] (broadcast over OUTER h)
                    rb = rden_b[:, :, None, :].to_broadcast([C, 2, H, W])
                    nc.vector.tensor_tensor(
                        out=ep4, in0=ep4, in1=rb, op=OP.mult
                    )

                with tc.tile_pool(name="projps", bufs=2, space="PSUM") as projps:
                    for g in range(NQG):
                        proj_group(projps, wq2, bq2, q, "scalar", g)
                        proj_group(projps, wk2, bk2, k, "scalar", g)
                    attn_iter(0)
                    for g in range(NQG):
                        proj_group(projps, wv2, bv2, v, "vector", g)
                    attn_iter(1)

                with tc.tile_pool(name="pvtps", bufs=2, space="PSUM") as pvtps:
                    for grp in range(0, NCHUNK, 4):
                        cnt = min(4, NCHUNK - grp)
                        pvt = pvtps.tile([C, 512], BF16, tag="pvt")
                        for j in range(cnt):
                            ch = grp + j
                            nc.tensor.transpose(
                                pvt[:, 128 * j : 128 * (j + 1)],
                                v[:, 128 * ch : 128 * (ch + 1)],
                                ident,
                            )
                        nc.vector.tensor_copy(
                            vT[:, 128 * grp : 128 * (grp + cnt)], pvt[:, : 128 * cnt]
                        )
                    attn_iter(2)
                    attn_iter(3)

                with tc.tile_pool(name="ctxps", bufs=1, space="PSUM") as ctxps:
                    ctx_ps.extend(
                        ctxps.tile([C, 512], F32, tag=f"ctx{i}", name=f"ctx_ps{i}")
                        for i in range(len(CTX_LIVE))
                    )
                    for it in range(4, NCHUNK + 4):
                        attn_iter(it)

                    for i, o in enumerate(CTX_LIVE):
                        if i % 2 == 0:
                            nc.scalar.copy(ctx_all[:, o : o + 512], ctx_ps[i][:, :])
                        else:
                            nc.vector.tensor_copy(
                                ctx_all[:, o : o + 512], ctx_ps[i][:, :]
                            )

            # ---- ctx tail (columns 2048:2304) + output projection + residual.
            # ---- TensorE queue order: outproj g0/g1 first (their ctx is ready
            # ---- at the last AV), the 18-matmul tail chain after, g2 last ----
            out_nat = data.tile([C, N], F32)
            with tc.tile_pool(name="ops", bufs=2, space="PSUM") as ops:
                def outproj(g):
                    po = ops.tile([C, QG], F32, tag="po", name=f"po_{g}")
                    o = g * QG
                    nc.tensor.matmul(
                        po[:, 0:512], wmt[:, 0:128], wmt[:],
                        start=True, stop=True,
                    )
                    nc.tensor.matmul(
                        po[:, 0:512], woTb[:], ctx_all[:, o : o + 512],
                        start=True, stop=True,
                    )
                    nc.tensor.matmul(
                        po[:, 512:QG], woTb[:], ctx_all[:, o + 512 : o + QG],
                        start=True, stop=True,
                    )
                    # fused bias + residual: out = (po + bo) + x
                    nc.vector.scalar_tensor_tensor(
                        out_nat[:, o : o + QG], po[:], bo,
                        txf[:, o : o + QG], op0=OP.add, op1=OP.add,
                    )
                    nc.sync.dma_start(
                        out_d[:, o : o + QG], out_nat[:, o : o + QG]
                    )

                outproj(0)
                outproj(1)
                tail = ops.tile([C, TAIL_SZ], F32, tag="tail")
                SPLIT_LAST = True
                nc.tensor.matmul(
                    tail[:, :], wmt[:, 0:128], wmt[:, 0:TAIL_SZ],
                    start=True, stop=True,
                )
                for ch in range(NCHUNK):
                    nc.tensor.matmul(
                        tail[:, :],
                        vT[:, 128 * ch : 128 * (ch + 1)],
                        e_tiles[ch][:, TAIL_OFF : TAIL_OFF + TAIL_SZ],
                        start=(ch == 0),
                        stop=(ch == NCHUNK - 1),
                    )
                nc.scalar.copy(ctx_all[:, TAIL_OFF : TAIL_OFF + TAIL_SZ], tail[:, :])
                # last group: halve the evac+DMA so the final DMA starts sooner
                po = ops.tile([C, QG], F32, tag="po", name="po_2")
                o = 2 * QG
                nc.tensor.matmul(
                    po[:, 0:512], wmt[:, 0:128], wmt[:],
                    start=True, stop=True,
                )
                nc.tensor.matmul(
                    po[:, 0:512], woTb[:], ctx_all[:, o : o + 512],
                    start=True, stop=True,
                )
                nc.tensor.matmul(
                    po[:, 512:QG], woTb[:], ctx_all[:, o + 512 : o + QG],
                    start=True, stop=True,
                )
                for half in range(2):
                    ho = o + 384 * half
                    nc.vector.scalar_tensor_tensor(
                        out_nat[:, ho : ho + 384], po[:, 384 * half : 384 * (half + 1)],
                        bo, txf[:, ho : ho + 384], op0=OP.add, op1=OP.add,
                    )
                    nc.sync.dma_start(
                        out_d[:, ho : ho + 384], out_nat[:, ho : ho + 384]
                    )

    nc.compile()
    return nc


_PROGRAM_CACHE = None


def make_in_maps(inputs):
    """Build the per-core input dicts (constants packed into one blob)."""
    import ml_dtypes

    f32 = lambda a: np.ascontiguousarray(np.asarray(a), dtype=np.float32)
    x = f32(inputs["x"])
    scale = 1.0 / np.sqrt(np.float32(C))

    gmat = np.zeros((C, GROUPS), np.float32)
    gmat[np.arange(C), np.arange(C) // GSIZE] = 1.0

    CB = 4 * C + GROUPS + C // 2 + 6
    cblob = np.zeros((C, CB), np.float32)
    cblob[:, 0:C] = f32(inputs["wq"]).T * scale
    cblob[:, C : 2 * C] = f32(inputs["wk"]).T
    cblob[:, 2 * C : 3 * C] = f32(inputs["wv"]).T
    cblob[:, 3 * C : 4 * C] = f32(inputs["wo"]).T
    cblob[:, 4 * C : 4 * C + GROUPS] = gmat
    ident = np.eye(C).astype(ml_dtypes.bfloat16)
    cblob[:, 4 * C + GROUPS : 4 * C + GROUPS + C // 2] = (
        np.ascontiguousarray(ident).view(np.float32)
    )
    bof = 4 * C + GROUPS + C // 2
    for j, (name, sc) in enumerate((
        ("gn_w", 1.0), ("gn_b", 1.0), ("bq", scale),
        ("bk", 1.0), ("bv", 1.0), ("bo", 1.0),
    )):
        cblob[:, bof + j] = f32(inputs[name]).reshape(C) * sc

    shared = {
        "cblob": cblob,
        "gexp": np.ascontiguousarray(gmat.T),
        "woTb": np.ascontiguousarray(
            f32(inputs["wo"]).T.astype(ml_dtypes.bfloat16)
        ),
    }
    return [
        {**shared, "x": np.ascontiguousarray(x[b].reshape(C, N))} for b in range(B)
    ]


def kernel(**inputs: np.ndarray) -> np.ndarray:
    global _PROGRAM_CACHE
    if _PROGRAM_CACHE is None:
        _PROGRAM_CACHE = _build_program()
    nc = _PROGRAM_CACHE

    in_maps = make_in_maps(inputs)
    res = bass_utils.run_bass_kernel_spmd(nc, in_maps, core_ids=list(range(NCORES)))
    out = np.stack([res.results[b]["out"].reshape(C, H, W) for b in range(B)])
    return out.astype(np.float32)
